# revision 8
# baseline (speedup 1.0000x reference)
import sys
sys.path.insert(0, '/opt/trn_rl_repo')
import numpy as np
import concourse.bass as bass
import concourse.bacc as bacc
import concourse.tile as tile
import concourse.mybir as mybir
from concourse import bass_utils

B, N, D, E = 32, 128, 64, 16384
NCORE = 8
NI = N // NCORE        # 16 i-rows per core
NE = NI * N            # 2048 edges per core
EPS = 1e-5
F32 = mybir.dt.float32
AF = mybir.ActivationFunctionType
ALU = mybir.AluOpType
IG = 4                 # i-tiles per psum group

_built = {}


def _body(tc):
    nc = tc.nc
    din = lambda n, sh: nc.dram_tensor(n, sh, F32, kind="ExternalInput").ap()
    xT = din("xT", [B, D, N])          # host-transposed x
    xsT = din("xsT", [B, D, NI])       # core's x rows, transposed
    xsn = din("xsn", [NI, B, D])       # core's x rows, (i,b,d)
    edge = din("edge", [B, NE, D])
    ident = din("ident", [N, N])
    ones_r = din("ones_r", [1, N])
    ones_c = din("ones_c", [N, 1])
    Wz, Wn, Wb, We, gv, bv, ge, be = {}, {}, {}, {}, {}, {}, {}, {}
    for l in (1, 2):
        Wz[l] = din(f"wz{l}", [D, D])
        Wn[l] = din(f"wn{l}", [D, D + 1])
        Wb[l] = din(f"wb{l}", [D, D + 1])
        We[l] = din(f"we{l}", [D, D + 1])
        gv[l] = din(f"gv{l}", [NI, 1])
        bv[l] = din(f"bv{l}", [NI, 1])
        ge[l] = din(f"ge{l}", [N, NI])
        be[l] = din(f"be{l}", [N, NI])
    x_out = nc.dram_tensor("x_out", [B, NI, D], F32, kind="ExternalOutput").ap()
    e_out = nc.dram_tensor("e_out", [B, NE, D], F32, kind="ExternalOutput").ap()


    with (
        tc.tile_pool(name="zb", bufs=1) as zb,
        tc.tile_pool(name="cst", bufs=1) as cst,
        tc.tile_pool(name="res", bufs=1) as res,
        tc.tile_pool(name="wrk", bufs=2) as wrk,
        tc.tile_pool(name="nod", bufs=1) as nod,
        tc.tile_pool(name="ps", bufs=2, space="PSUM") as ps,
        tc.tile_pool(name="pse", bufs=2, space="PSUM") as pse,
        tc.tile_pool(name="dr", bufs=1, space="DRAM") as dr,
        tc.tile_pool(name="bdp", bufs=1) as bdp,
    ):
        Z = zb.tile([N, NI, B, D], F32)
        Zae = zb.tile([N, NI, B], F32)
        ag_in = dr.tile([NI, B, D], F32)
        ag_out = dr.tile([N, B, D], F32, addr_space="Shared")              # 17MB resident
        idn = cst.tile([N, N], F32, name="idn"); nc.sync.dma_start(idn[:], ident[:])
        ons = cst.tile([1, N], F32, name="ons"); nc.sync.dma_start(ons[:], ones_r[:])
        onc = cst.tile([N, 1], F32, name="onc"); nc.sync.dma_start(onc[:], ones_c[:])
        WzS, WnS, WbS, WeS = {}, {}, {}, {}
        geS, beS, gvS, bvS = {}, {}, {}, {}
        for l in (1, 2):
            WzS[l] = cst.tile([D, D], F32, name=f"wzs{l}"); nc.sync.dma_start(WzS[l][:], Wz[l][:])
            WnS[l] = cst.tile([D, D + 1], F32, name=f"wns{l}"); nc.sync.dma_start(WnS[l][:], Wn[l][:])
            WbS[l] = cst.tile([D, D + 1], F32, name=f"wbs{l}"); nc.sync.dma_start(WbS[l][:], Wb[l][:])
            WeS[l] = cst.tile([D, D + 1], F32, name=f"wes{l}"); nc.sync.dma_start(WeS[l][:], We[l][:])
            geS[l] = cst.tile([N, NI], F32, name=f"ges{l}"); nc.sync.dma_start(geS[l][:], ge[l][:])
            beS[l] = cst.tile([N, NI], F32, name=f"bes{l}"); nc.sync.dma_start(beS[l][:], be[l][:])
            gvS[l] = cst.tile([NI, 1], F32, name=f"gvs{l}"); nc.sync.dma_start(gvS[l][:], gv[l][:])
            bvS[l] = cst.tile([NI, 1], F32, name=f"bvs{l}"); nc.sync.dma_start(bvS[l][:], bv[l][:])
        xTr = res.tile([D, B, N], F32, name="xTr"); nc.sync.dma_start(xTr[:], xT.rearrange("b d n -> d b n"))
        xsTr = res.tile([D, B, NI], F32, name="xsTr"); nc.sync.dma_start(xsTr[:], xsT.rearrange("b d n -> d b n"))
        xin1 = res.tile([NI, B, D], F32, name="xin1"); nc.sync.dma_start(xin1[:], xsn[:])
        xnew = res.tile([NI, B, D], F32)
        Agg = res.tile([NI, B, D], F32)

        for l in (1, 2):
            for b in range(B):
                # node-side lhsT tensors for this b
                if l == 1:
                    xTb, xsTb = xTr[:, b, :], xsTr[:, b, :]
                else:
                    x2n = wrk.tile([N, D], F32, tag="x2n")
                    nc.sync.dma_start(x2n[:], ag_out[:, b, :])
                    p1 = ps.tile([D, N], F32, tag="pn")
                    nc.tensor.transpose(p1[:], x2n[:], idn[:])
                    xTb_t = wrk.tile([D, N], F32, tag="xtb")
                    nc.scalar.copy(out=xTb_t[:], in_=p1[:])
                    p2 = ps.tile([D, NI], F32, tag="pn")
                    nc.tensor.transpose(p2[:], xnew[:, b, :], idn[:NI, :NI])
                    xsTb_t = wrk.tile([D, NI], F32, tag="xstb")
                    nc.scalar.copy(out=xsTb_t[:], in_=p2[:])
                    xTb, xsTb = xTb_t[:], xsTb_t[:]
                pz = ps.tile([N, D], F32, tag="pn")
                nc.tensor.matmul(pz[:], xTb, WzS[l][:], start=True, stop=True)
                zh = wrk.tile([N, D], F32, tag="zh")
                nc.scalar.copy(out=zh[:], in_=pz[:])
                pb = ps.tile([NI, D + 1], F32, tag="pn")
                nc.tensor.matmul(pb[:], xsTb, WbS[l][:], start=True, stop=True)
                bd = wrk.tile([NI, D + 1], F32, tag="bd")
                nc.scalar.copy(out=bd[:], in_=pb[:])
                bd1 = bdp.tile([1, NI * (D + 1)], F32, tag="bd1")
                nc.sync.dma_start(bd1[:], bd[:])
                # edge tiles
                if l == 1:
                    esb = wrk.tile([N, NI, D], F32, tag="esb")
                    nc.sync.dma_start(esb[:], edge[b].rearrange("(i j) d -> j i d", j=N))
                for g in range(NI // IG):
                    pz4 = pse.tile([N, IG, D + 1], F32, tag="pz4")
                    for k in range(IG):
                        i = g * IG + k
                        src = esb[:, i, :] if l == 1 else Z[:, i, b, :]
                        pet = pse.tile([D, N], F32, tag="pet")
                        nc.tensor.transpose(pet[:], src, idn[:])
                        ets = wrk.tile([D, N], F32, tag="ets")
                        nc.scalar.copy(out=ets[:], in_=pet[:])
                        nc.tensor.matmul(pz4[:, k, :], ets[:], WeS[l][:], start=True, stop=False)
                        nc.tensor.matmul(pz4[:, k, :], xTb, WnS[l][:], start=False, stop=False)
                        nc.tensor.matmul(pz4[:, k, :], ons[:], bd1[:, i * (D + 1):(i + 1) * (D + 1)], start=False, stop=True)
                    nc.scalar.copy(out=Z[:, g * IG:(g + 1) * IG, b, :], in_=pz4[:, :, 0:D])
                    nc.scalar.copy(out=Zae[:, g * IG:(g + 1) * IG, b], in_=pz4[:, :, D])
                # attention + aggregation for this b
                lr = nod.tile([N, NI], F32, tag="lr")
                nc.scalar.activation(lr[:], Zae[:, :, b], AF.Lrelu, alpha=0.01)
                ex = nod.tile([N, NI], F32, tag="ex")
                nc.scalar.activation(ex[:], lr[:], AF.Exp)
                psm = ps.tile([NI, 1], F32, tag="pn")
                nc.tensor.matmul(psm[:], ex[:], onc[:], start=True, stop=True)
                rcp = nod.tile([NI, 1], F32, tag="rcp")
                nc.vector.reciprocal(rcp[:], psm[:])
                pat = ps.tile([D, NI], F32, tag="pn")
                nc.tensor.matmul(pat[:], zh[:], ex[:], start=True, stop=True)
                ats = nod.tile([D, NI], F32, tag="ats")
                nc.scalar.copy(out=ats[:], in_=pat[:])
                pan = ps.tile([NI, D], F32, tag="pn")
                nc.tensor.transpose(pan[:], ats[:], idn[:D, :D])
                nc.vector.tensor_scalar(Agg[:, b, :], pan[:], rcp[:], None, op0=ALU.mult)

            # ---- edge BN (stats over (b,d) per edge-channel) + ELU, in place
            for i in range(NI):
                st = nod.tile([N, 4, 6], F32, tag="st")
                zi = Z[:, i].rearrange("p b d -> p (b d)")
                for c in range(4):
                    nc.vector.bn_stats(st[:, c, :], zi[:, c * 512:(c + 1) * 512])
                mv = nod.tile([N, 2], F32, tag="mv")
                nc.vector.bn_aggr(mv[:], st[:])
                rs = nod.tile([N, 1], F32, tag="rs")
                nc.vector.tensor_scalar(rs[:], mv[:, 1:2], EPS, None, op0=ALU.add)
                sq = nod.tile([N, 1], F32, tag="sq")
                nc.scalar.activation(sq[:], rs[:], AF.Sqrt)
                nc.vector.reciprocal(rs[:], sq[:])
                s1 = nod.tile([N, 1], F32, tag="s1")
                nc.vector.tensor_tensor(s1[:], rs[:], geS[l][:, i:i + 1], op=ALU.mult)
                s2 = nod.tile([N, 1], F32, tag="s2")
                nc.vector.tensor_tensor(s2[:], mv[:, 0:1], s1[:], op=ALU.mult)
                nc.vector.tensor_tensor(s2[:], beS[l][:, i:i + 1], s2[:], op=ALU.subtract)
                for h in range(4):
                    zs = Z[:, i, h * 8:(h + 1) * 8, :]
                    w = nod.tile([N, 8, D], F32, tag="w")
                    nc.scalar.activation(w[:], zs, AF.Exp, bias=s2[:], scale=s1[:])
                    y = nod.tile([N, 8, D], F32, tag="y")
                    nc.vector.tensor_scalar(y[:], zs, s1[:], s2[:], op0=ALU.mult, op1=ALU.add)
                    nc.vector.tensor_scalar(w[:], w[:], 1.0, -1.0, op0=ALU.min, op1=ALU.add)
                    nc.vector.tensor_tensor(zs, y[:], w[:], op=ALU.max)

            # ---- node BN + residual + ELU
            nst = nod.tile([NI, 4, 6], F32, tag="nst")
            agf = Agg[:].rearrange("p b d -> p (b d)")
            for c in range(4):
                nc.vector.bn_stats(nst[:, c, :], agf[:, c * 512:(c + 1) * 512])
            nmv = nod.tile([NI, 2], F32, tag="nmv")
            nc.vector.bn_aggr(nmv[:], nst[:])
            nrs = nod.tile([NI, 1], F32, tag="nrs")
            nc.vector.tensor_scalar(nrs[:], nmv[:, 1:2], EPS, None, op0=ALU.add)
            nsq = nod.tile([NI, 1], F32, tag="nsq")
            nc.scalar.activation(nsq[:], nrs[:], AF.Sqrt)
            nc.vector.reciprocal(nrs[:], nsq[:])
            n1 = nod.tile([NI, 1], F32, tag="n1")
            nc.vector.tensor_tensor(n1[:], nrs[:], gvS[l][:], op=ALU.mult)
            n2 = nod.tile([NI, 1], F32, tag="n2")
            nc.vector.tensor_tensor(n2[:], nmv[:, 0:1], n1[:], op=ALU.mult)
            nc.vector.tensor_tensor(n2[:], bvS[l][:], n2[:], op=ALU.subtract)
            xprev = xin1 if l == 1 else xnew
            dst = xnew if l == 1 else xin1
            for h in range(2):
                hs = slice(h * 16, (h + 1) * 16)
                yr = nod.tile([NI, 16, D], F32, tag="yr")
                nc.vector.tensor_scalar(yr[:], Agg[:, hs, :], n1[:], n2[:], op0=ALU.mult, op1=ALU.add)
                nc.vector.tensor_tensor(yr[:], yr[:], xprev[:, hs, :], op=ALU.add)
                wn = nod.tile([NI, 16, D], F32, tag="wn")
                nc.scalar.activation(wn[:], yr[:], AF.Exp)
                nc.vector.tensor_scalar(wn[:], wn[:], 1.0, -1.0, op0=ALU.min, op1=ALU.add)
                nc.vector.tensor_tensor(dst[:, hs, :], yr[:], wn[:], op=ALU.max)

            if l == 1:
                nc.sync.dma_start(ag_in[:], xnew[:])
                nc.gpsimd.collective_compute(
                    "AllGather", ALU.bypass,
                    ins=[ag_in[:]], outs=[ag_out[:]],
                    replica_groups=[list(range(NCORE))],
                )

        for b in range(B):
            nc.sync.dma_start(e_out[b].rearrange("(i j) d -> j i d", j=N), Z[:, :, b, :])
        nc.sync.dma_start(x_out.rearrange("b i d -> i b d"), xin1[:])


def _build():
    if "nc" in _built:
        return _built["nc"]
    nc = bacc.Bacc("TRN2", target_bir_lowering=False, debug=False, num_devices=NCORE)
    with tile.TileContext(nc) as tc:
        _body(tc)
    nc.compile()
    _built["nc"] = nc
    return nc


def kernel(x, edge, Wh1, We1, Wp1, Wa1, Wh2, We2, Wp2, Wa2,
           gv1, bv1, ge1, be1, gv2, bv2, ge2, be2, **_):
    x = np.asarray(x, np.float32); edge = np.asarray(edge, np.float32)
    nc = _build()
    com = {
        "xT": np.ascontiguousarray(x.transpose(0, 2, 1)),
        "ident": np.eye(N, dtype=np.float32),
        "ones_r": np.ones((1, N), np.float32),
        "ones_c": np.ones((N, 1), np.float32),
    }
    for l, (Wh, Wee, Wp, Wa, gvv, bvv, gee, bee) in {
        1: (Wh1, We1, Wp1, Wa1, gv1, bv1, ge1, be1),
        2: (Wh2, We2, Wp2, Wa2, gv2, bv2, ge2, be2),
    }.items():
        Wh, Wee, Wp, Wa = (np.asarray(a, np.float32) for a in (Wh, Wee, Wp, Wa))
        Wps, Wpd, Wpe = Wp[:, 0:D], Wp[:, D:2 * D], Wp[:, 2 * D:3 * D]
        was, wad, wae = Wa[0, 0:D], Wa[0, D:2 * D], Wa[0, 2 * D:3 * D]
        com[f"wz{l}"] = np.ascontiguousarray(Wh.T)
        com[f"wn{l}"] = np.concatenate([(Wps @ Wh).T, (Wh.T @ was)[:, None]], 1)
        com[f"wb{l}"] = np.concatenate([(Wpd @ Wh).T, (Wh.T @ wad)[:, None]], 1)
        com[f"we{l}"] = np.concatenate([(Wpe @ Wee).T, (Wee.T @ wae)[:, None]], 1)
        com[f"gv{l}"] = np.zeros((NI, 1), np.float32)
        com[f"bv{l}"] = np.zeros((NI, 1), np.float32)
        com[f"ge{l}"] = np.zeros((N, NI), np.float32)
        com[f"be{l}"] = np.zeros((N, NI), np.float32)
    in_maps = []
    for c in range(NCORE):
        m = dict(com)
        sl = slice(c * NI, (c + 1) * NI)
        esl = slice(c * NE, (c + 1) * NE)
        m["xsT"] = np.ascontiguousarray(x[:, sl, :].transpose(0, 2, 1))
        m["xsn"] = np.ascontiguousarray(x[:, sl, :].transpose(1, 0, 2))
        m["edge"] = np.ascontiguousarray(edge[:, esl, :])
        for l, (gvv, bvv, gee, bee) in {1: (gv1, bv1, ge1, be1), 2: (gv2, bv2, ge2, be2)}.items():
            m[f"gv{l}"] = np.asarray(gvv, np.float32)[sl].reshape(NI, 1)
            m[f"bv{l}"] = np.asarray(bvv, np.float32)[sl].reshape(NI, 1)
            m[f"ge{l}"] = np.ascontiguousarray(np.asarray(gee, np.float32)[esl].reshape(NI, N).T)
            m[f"be{l}"] = np.ascontiguousarray(np.asarray(bee, np.float32)[esl].reshape(NI, N).T)
        in_maps.append(m)
    res = bass_utils.run_bass_kernel_spmd(nc, in_maps, core_ids=list(range(NCORE)))
    xo = np.concatenate([res.results[c]["x_out"] for c in range(NCORE)], axis=1)
    eo = np.concatenate([res.results[c]["e_out"] for c in range(NCORE)], axis=1)
    return xo, eo
